# revision 9
# baseline (speedup 1.0000x reference)
"""Trainium2 Bass kernel for a single-step decoder LSTM (V=50257, H=1024).

Computation (per reference):
    x = relu(emb[token]); gates = x@W_ih.T + b_ih + h0@W_hh.T + b_hh
    i,f,g,o = split(gates); c1 = sig(f)*c0 + sig(i)*tanh(g)
    h1 = sig(o)*tanh(c1); out = log_softmax(h1@W_out.T + b_out)

Sharding (8 cores, tensor-parallel):
  - LSTM gate rows sharded by hidden block: core k owns units k*128..+128,
    reads only its 4x128 rows of W_ih/W_hh (fp32 -> h1/c1 outputs exact).
  - h1 shards AllGather'd on-device (f16, 2KB).
  - W_out sharded by vocab rows (6283/core, fp16 on the wire: weight
    rounding adds ~3e-5 rel err to the log-softmax output), streamed
    through SBUF; per-row dot = DVE multiply + ScalarE Identity+accum.
  - softmax denominator: ScalarE Exp+accum, cross-partition sum via a
    PE ones-matmul, per-core sums AllGather'd, each core normalizes its
    shard.  Embedding lookup is a host-side row gather (4KB of 206MB).

Numerics: logits are O(0.5), so exp() without max-subtraction is safe;
padded vocab rows get bias -1e30 => exp==0, never the max.
"""

import sys
import numpy as np

for _p in ("/opt/trn_rl_repo",):
    if _p not in sys.path:
        sys.path.insert(0, _p)

V = 50257
H = 1024
NCORES = 8
VPC = 6283              # ceil(V / 8) vocab rows per core
VTILES = 50             # ceil(VPC / 128)
VPAD = VTILES * 128     # 6400
NEG = -1.0e30

_NC = None
LAST_RESULTS = None


def _build_nc():
    import concourse.bacc as bacc
    import concourse.tile as tile
    from concourse import mybir

    f32 = mybir.dt.float32
    f16 = mybir.dt.float16
    AF = mybir.ActivationFunctionType
    X = mybir.AxisListType.X
    rg = [list(range(NCORES))]

    nc = bacc.Bacc("TRN2", target_bir_lowering=False, debug=False,
                   num_devices=NCORES)

    xh_d = nc.dram_tensor("xh", [1, 2 * H], f32, kind="ExternalInput").ap()
    wcat_d = nc.dram_tensor("wcat", [4, 128, 2 * H], f32,
                            kind="ExternalInput").ap()
    bsum_d = nc.dram_tensor("bsum", [128, 4], f32, kind="ExternalInput").ap()
    cblk_d = nc.dram_tensor("cblk", [128, 1], f32, kind="ExternalInput").ap()
    wout_d = nc.dram_tensor("wout", [VPAD, H], f16, kind="ExternalInput").ap()
    bout_d = nc.dram_tensor("bout", [128, VTILES], f32,
                            kind="ExternalInput").ap()
    out_d = nc.dram_tensor("out_shard", [128, VTILES], f32,
                           kind="ExternalOutput").ap()
    h1_d = nc.dram_tensor("h1_out", [128, 1], f32, kind="ExternalOutput").ap()
    c1_d = nc.dram_tensor("c1_out", [128, 1], f32, kind="ExternalOutput").ap()

    with tile.TileContext(nc) as tc:
        with (
            tc.tile_pool(name="dram", bufs=1, space="DRAM") as dpool,
            tc.tile_pool(name="psum", bufs=1, space="PSUM") as psum,
            tc.tile_pool(name="consts", bufs=1) as consts,
            tc.tile_pool(name="wpool", bufs=14) as wpool,
            tc.tile_pool(name="scratch", bufs=2) as scratch,
            tc.tile_pool(name="small", bufs=1) as small,
        ):
            # xh first: it gates every gate-mul; tiny consts next; LSTM
            # weights after; W_out tiles stream behind them.
            xh = consts.tile([128, 2 * H], f32, name="xh_t", tag="xh_t")
            nc.sync.dma_start(out=xh, in_=xh_d.to_broadcast([128, 2 * H]))
            nc.scalar.activation(out=xh[:, 0:H], in_=xh[:, 0:H], func=AF.Relu)
            bsum = consts.tile([128, 4], f32, name="bsum_t", tag="bsum_t")
            nc.sync.dma_start(out=bsum, in_=bsum_d)
            cblk = consts.tile([128, 1], f32, name="cblk_t", tag="cblk_t")
            nc.sync.dma_start(out=cblk, in_=cblk_d)
            bout = consts.tile([128, VTILES], f32, name="bout_t", tag="bout_t")
            nc.sync.dma_start(out=bout, in_=bout_d)
            ones = consts.tile([128, 1], f32, name="ones_t", tag="ones_t")
            nc.vector.memset(ones, 1.0)
            wg = []
            for g in range(4):
                t = consts.tile([128, 2 * H], f32, name=f"wg{g}", tag=f"wg{g}")
                # two half-loads so the first gate-mul starts ~3us earlier
                nc.sync.dma_start(out=t[:, 0:H], in_=wcat_d[g, :, 0:H])
                nc.sync.dma_start(out=t[:, H:2 * H], in_=wcat_d[g, :, H:2 * H])
                wg.append(t)

            # gates: 8 half-width DVE multiplies + ScalarE row-sums, then
            # pair-add; finer pieces pipeline tighter with the wcat DMAs.
            graw2 = small.tile([128, 8], f32, name="graw2", tag="graw2")
            for piece in range(8):
                g, half = piece // 2, piece % 2
                ps = scratch.tile([128, H], f32, name=f"lp{piece}",
                                  tag="prod")
                nc.vector.tensor_mul(ps, wg[g][:, half * H:(half + 1) * H],
                                     xh[:, half * H:(half + 1) * H])
                nc.scalar.activation(out=ps, in_=ps, func=AF.Identity,
                                     accum_out=graw2[:, piece:piece + 1])
            graw = small.tile([128, 4], f32, name="graw", tag="graw")
            nc.vector.tensor_add(graw, graw2[:, 0:8:2], graw2[:, 1:8:2])
            gates = small.tile([128, 4], f32, name="gates", tag="gates")
            nc.vector.tensor_add(gates, graw, bsum)

            sig_if = small.tile([128, 2], f32, name="sig_if", tag="sig_if")
            nc.scalar.activation(out=sig_if, in_=gates[:, 0:2],
                                 func=AF.Sigmoid)
            tanh_g = small.tile([128, 1], f32, name="tanh_g", tag="tanh_g")
            nc.scalar.activation(out=tanh_g, in_=gates[:, 2:3], func=AF.Tanh)
            sig_o = small.tile([128, 1], f32, name="sig_o", tag="sig_o")
            nc.scalar.activation(out=sig_o, in_=gates[:, 3:4],
                                 func=AF.Sigmoid)
            t1 = small.tile([128, 1], f32, name="t1", tag="t1")
            nc.vector.tensor_mul(t1, sig_if[:, 1:2], cblk)
            t2 = small.tile([128, 1], f32, name="t2", tag="t2")
            nc.vector.tensor_mul(t2, sig_if[:, 0:1], tanh_g)
            c1 = small.tile([128, 1], f32, name="c1", tag="c1")
            nc.vector.tensor_add(c1, t1, t2)
            tc1 = small.tile([128, 1], f32, name="tc1", tag="tc1")
            nc.scalar.activation(out=tc1, in_=c1, func=AF.Tanh)
            h1 = small.tile([128, 1], f32, name="h1", tag="h1")
            nc.vector.tensor_mul(h1, sig_o, tc1)
            nc.sync.dma_start(out=h1_d, in_=h1)
            nc.sync.dma_start(out=c1_d, in_=c1)

            # h1 -> f16, AllGather shards, broadcast full h1 to partitions
            h1w = small.tile([128, 1], f16, name="h1w", tag="h1w")
            nc.vector.tensor_copy(h1w, h1)
            h1_send = dpool.tile([128, 1], f16, name="h1_send", tag="h1_send")
            nc.sync.dma_start(out=h1_send, in_=h1w)
            h1_all = dpool.tile([1, NCORES * 128], f16, name="h1_all",
                                tag="h1_all")
            nc.gpsimd.collective_compute(
                "AllGather", mybir.AluOpType.bypass, replica_groups=rg,
                ins=[h1_send.opt()], outs=[h1_all.opt()])
            h1b = consts.tile([128, H], f16, name="h1b", tag="h1b")
            nc.sync.dma_start(out=h1b, in_=h1_all.to_broadcast([128, H]))

            # logits shard: fp16 W_out stream; DVE multiplies then folds the
            # 1024-wide product to 512 with one add, so ScalarE's accum (no
            # 16-bit speedup) only reads half — balances DVE/ACT under the
            # DMA rate.
            lraw = consts.tile([128, VTILES], f32, name="lraw", tag="lraw")
            for n in range(VTILES):
                wt = wpool.tile([128, H], f16, name=f"wt{n}", tag="wt")
                nc.sync.dma_start(out=wt, in_=wout_d[n * 128:(n + 1) * 128, :])
                ps = scratch.tile([128, H], f16, name=f"wprod{n}", tag="wprod")
                nc.vector.tensor_mul(ps, wt, h1b)
                ps2 = scratch.tile([128, H // 2], f16, name=f"wh{n}",
                                   tag="whalf")
                nc.vector.tensor_add(ps2, ps[:, 0:H // 2], ps[:, H // 2:H])
                nc.scalar.activation(out=ps2, in_=ps2, func=AF.Identity,
                                     accum_out=lraw[:, n:n + 1])
            logits = consts.tile([128, VTILES], f32, name="logits",
                                 tag="logits")
            nc.vector.tensor_add(logits, lraw, bout)

            # softmax denominator; cross-partition sum via PE ones-matmul
            e = scratch.tile([128, VTILES], f32, name="e_t", tag="e_t")
            s_p = small.tile([128, 1], f32, name="s_p", tag="s_p")
            nc.scalar.activation(out=e, in_=logits, func=AF.Exp,
                                 accum_out=s_p)
            s_psum = psum.tile([1, 1], f32, name="s_psum", tag="s_psum")
            nc.tensor.matmul(s_psum, s_p, ones)
            S_loc = small.tile([1, 1], f32, name="S_loc", tag="S_loc")
            nc.scalar.copy(out=S_loc, in_=s_psum)

            st_send = dpool.tile([1, 1], f32, name="st_send", tag="st_send")
            nc.sync.dma_start(out=st_send, in_=S_loc)
            st_all = dpool.tile([1, NCORES], f32, name="st_all", tag="st_all")
            nc.gpsimd.collective_compute(
                "AllGather", mybir.AluOpType.bypass, replica_groups=rg,
                ins=[st_send.opt()], outs=[st_all.opt()])
            sa = small.tile([128, NCORES], f32, name="sa", tag="sa")
            nc.sync.dma_start(out=sa, in_=st_all.to_broadcast([128, NCORES]))
            S_g = small.tile([128, 1], f32, name="S_g", tag="S_g")
            nc.vector.reduce_sum(out=S_g, in_=sa, axis=X)
            logZ = small.tile([128, 1], f32, name="logZ", tag="logZ")
            nc.scalar.activation(out=logZ, in_=S_g, func=AF.Ln)
            outn = scratch.tile([128, VTILES], f32, name="outn", tag="outn")
            nc.vector.tensor_scalar(out=outn, in0=logits, scalar1=logZ,
                                    scalar2=None,
                                    op0=mybir.AluOpType.subtract)
            nc.sync.dma_start(out=out_d, in_=outn)

    nc.compile()
    return nc


def _prep_in_maps(inputs):
    token = int(np.asarray(inputs["token"]).reshape(-1)[0])
    emb = np.asarray(inputs["emb"], dtype=np.float32)
    h0 = np.asarray(inputs["h0"], dtype=np.float32).reshape(-1)
    c0 = np.asarray(inputs["c0"], dtype=np.float32).reshape(-1)
    W_ih = np.asarray(inputs["W_ih"], dtype=np.float32).reshape(4, H, H)
    W_hh = np.asarray(inputs["W_hh"], dtype=np.float32).reshape(4, H, H)
    b = (np.asarray(inputs["b_ih"], dtype=np.float32)
         + np.asarray(inputs["b_hh"], dtype=np.float32)).reshape(4, H)
    W_out = np.asarray(inputs["W_out"], dtype=np.float32)
    b_out = np.asarray(inputs["b_out"], dtype=np.float32)

    xh = np.concatenate([emb[token], h0])[None, :]  # [1, 2H]

    in_maps = []
    for k in range(NCORES):
        lo, hi = k * 128, (k + 1) * 128
        wcat = np.concatenate([W_ih[:, lo:hi, :], W_hh[:, lo:hi, :]],
                              axis=2)               # [4, 128, 2H]
        bsum = np.ascontiguousarray(b[:, lo:hi].T)  # [128, 4]
        cblk = c0[lo:hi].reshape(128, 1)

        vlo = k * VPC
        vhi = min(vlo + VPC, V)
        nv = vhi - vlo
        wout = np.zeros((VPAD, H), dtype=np.float16)
        wout[:nv] = W_out[vlo:vhi].astype(np.float16)
        bpad = np.full(VPAD, NEG, dtype=np.float32)
        bpad[:nv] = b_out[vlo:vhi]
        bout = np.ascontiguousarray(bpad.reshape(VTILES, 128).T)  # [128, VT]

        in_maps.append({
            "xh": np.ascontiguousarray(xh),
            "wcat": np.ascontiguousarray(wcat),
            "bsum": bsum,
            "cblk": np.ascontiguousarray(cblk),
            "wout": wout,
            "bout": bout,
        })
    return in_maps


def kernel(**inputs):
    global _NC, LAST_RESULTS
    if _NC is None:
        _NC = _build_nc()

    from concourse.bass_utils import run_bass_kernel_spmd

    in_maps = _prep_in_maps(inputs)
    res = run_bass_kernel_spmd(_NC, in_maps, core_ids=list(range(NCORES)))
    LAST_RESULTS = res

    out = np.empty((1, V), dtype=np.float32)
    h1 = np.empty(NCORES * 128, dtype=np.float32)
    c1 = np.empty(NCORES * 128, dtype=np.float32)
    for k in range(NCORES):
        r = res.results[k]
        vlo = k * VPC
        vhi = min(vlo + VPC, V)
        # out_shard[p, n] holds vocab row n*128 + p of this core's shard
        out[0, vlo:vhi] = r["out_shard"].T.reshape(-1)[:vhi - vlo]
        h1[k * 128:(k + 1) * 128] = r["h1_out"].reshape(-1)
        c1[k * 128:(k + 1) * 128] = r["c1_out"].reshape(-1)

    return out, h1.reshape(1, 1, H), c1.reshape(1, 1, H)


# revision 10
# speedup vs baseline: 1.0127x; 1.0127x over previous
"""Trainium2 Bass kernel for a single-step decoder LSTM (V=50257, H=1024).

Computation (per reference):
    x = relu(emb[token]); gates = x@W_ih.T + b_ih + h0@W_hh.T + b_hh
    i,f,g,o = split(gates); c1 = sig(f)*c0 + sig(i)*tanh(g)
    h1 = sig(o)*tanh(c1); out = log_softmax(h1@W_out.T + b_out)

Sharding (8 cores, tensor-parallel):
  - LSTM gate rows sharded by hidden block: core k owns units k*128..+128,
    reads only its 4x128 rows of W_ih/W_hh (fp32 -> h1/c1 outputs exact).
  - h1 shards AllGather'd on-device (f16, 2KB).
  - W_out sharded by vocab rows (6283/core, fp16 on the wire: weight
    rounding adds ~3e-5 rel err to the log-softmax output), streamed
    through SBUF; per-row dot = DVE multiply + ScalarE Identity+accum.
  - softmax denominator: ScalarE Exp+accum, cross-partition sum via a
    PE ones-matmul, per-core sums AllGather'd, each core normalizes its
    shard.  Embedding lookup is a host-side row gather (4KB of 206MB).

Numerics: logits are O(0.5), so exp() without max-subtraction is safe;
padded vocab rows get bias -1e30 => exp==0, never the max.
"""

import sys
import numpy as np

for _p in ("/opt/trn_rl_repo",):
    if _p not in sys.path:
        sys.path.insert(0, _p)

V = 50257
H = 1024
NCORES = 8
VPC = 6283              # ceil(V / 8) vocab rows per core
VTILES = 50             # ceil(VPC / 128)
VPAD = VTILES * 128     # 6400
NEG = -1.0e30

_NC = None
LAST_RESULTS = None


def _build_nc():
    import concourse.bacc as bacc
    import concourse.tile as tile
    from concourse import mybir

    f32 = mybir.dt.float32
    f16 = mybir.dt.float16
    AF = mybir.ActivationFunctionType
    X = mybir.AxisListType.X
    rg = [list(range(NCORES))]

    nc = bacc.Bacc("TRN2", target_bir_lowering=False, debug=False,
                   num_devices=NCORES)

    xh_d = nc.dram_tensor("xh", [1, 2 * H], f32, kind="ExternalInput").ap()
    wcat_d = nc.dram_tensor("wcat", [4, 128, 2 * H], f32,
                            kind="ExternalInput").ap()
    bsum_d = nc.dram_tensor("bsum", [128, 4], f32, kind="ExternalInput").ap()
    cblk_d = nc.dram_tensor("cblk", [128, 1], f32, kind="ExternalInput").ap()
    wout_d = nc.dram_tensor("wout", [VPAD, H], f16, kind="ExternalInput").ap()
    bout_d = nc.dram_tensor("bout", [128, VTILES], f32,
                            kind="ExternalInput").ap()
    out_d = nc.dram_tensor("out_shard", [128, VTILES], f32,
                           kind="ExternalOutput").ap()
    h1_d = nc.dram_tensor("h1_out", [128, 1], f32, kind="ExternalOutput").ap()
    c1_d = nc.dram_tensor("c1_out", [128, 1], f32, kind="ExternalOutput").ap()

    with tile.TileContext(nc) as tc:
        with (
            tc.tile_pool(name="dram", bufs=1, space="DRAM") as dpool,
            tc.tile_pool(name="psum", bufs=1, space="PSUM") as psum,
            tc.tile_pool(name="consts", bufs=1) as consts,
            tc.tile_pool(name="wpool", bufs=6) as wpool,
            tc.tile_pool(name="scratch", bufs=2) as scratch,
            tc.tile_pool(name="small", bufs=1) as small,
        ):
            # xh first: it gates every gate-mul; tiny consts next; LSTM
            # weights after; W_out tiles stream behind them.
            xh = consts.tile([128, 2 * H], f32, name="xh_t", tag="xh_t")
            nc.sync.dma_start(out=xh, in_=xh_d.to_broadcast([128, 2 * H]))
            nc.scalar.activation(out=xh[:, 0:H], in_=xh[:, 0:H], func=AF.Relu)
            bsum = consts.tile([128, 4], f32, name="bsum_t", tag="bsum_t")
            nc.sync.dma_start(out=bsum, in_=bsum_d)
            cblk = consts.tile([128, 1], f32, name="cblk_t", tag="cblk_t")
            nc.sync.dma_start(out=cblk, in_=cblk_d)
            bout = consts.tile([128, VTILES], f32, name="bout_t", tag="bout_t")
            nc.sync.dma_start(out=bout, in_=bout_d)
            ones = consts.tile([128, 1], f32, name="ones_t", tag="ones_t")
            nc.vector.memset(ones, 1.0)
            wg = []
            for g in range(4):
                t = consts.tile([128, 2 * H], f32, name=f"wg{g}", tag=f"wg{g}")
                # two half-loads so the first gate-mul starts ~3us earlier
                nc.sync.dma_start(out=t[:, 0:H], in_=wcat_d[g, :, 0:H])
                nc.sync.dma_start(out=t[:, H:2 * H], in_=wcat_d[g, :, H:2 * H])
                wg.append(t)

            # gates: 8 half-width DVE multiplies + ScalarE row-sums, then
            # pair-add; finer pieces pipeline tighter with the wcat DMAs.
            graw2 = small.tile([128, 8], f32, name="graw2", tag="graw2")
            for piece in range(8):
                g, half = piece // 2, piece % 2
                ps = scratch.tile([128, H], f32, name=f"lp{piece}",
                                  tag="prod")
                nc.vector.tensor_mul(ps, wg[g][:, half * H:(half + 1) * H],
                                     xh[:, half * H:(half + 1) * H])
                nc.scalar.activation(out=ps, in_=ps, func=AF.Identity,
                                     accum_out=graw2[:, piece:piece + 1])
            graw = small.tile([128, 4], f32, name="graw", tag="graw")
            nc.vector.tensor_add(graw, graw2[:, 0:8:2], graw2[:, 1:8:2])
            gates = small.tile([128, 4], f32, name="gates", tag="gates")
            nc.vector.tensor_add(gates, graw, bsum)

            sig_if = small.tile([128, 2], f32, name="sig_if", tag="sig_if")
            nc.scalar.activation(out=sig_if, in_=gates[:, 0:2],
                                 func=AF.Sigmoid)
            tanh_g = small.tile([128, 1], f32, name="tanh_g", tag="tanh_g")
            nc.scalar.activation(out=tanh_g, in_=gates[:, 2:3], func=AF.Tanh)
            sig_o = small.tile([128, 1], f32, name="sig_o", tag="sig_o")
            nc.scalar.activation(out=sig_o, in_=gates[:, 3:4],
                                 func=AF.Sigmoid)
            t1 = small.tile([128, 1], f32, name="t1", tag="t1")
            nc.vector.tensor_mul(t1, sig_if[:, 1:2], cblk)
            t2 = small.tile([128, 1], f32, name="t2", tag="t2")
            nc.vector.tensor_mul(t2, sig_if[:, 0:1], tanh_g)
            c1 = small.tile([128, 1], f32, name="c1", tag="c1")
            nc.vector.tensor_add(c1, t1, t2)
            tc1 = small.tile([128, 1], f32, name="tc1", tag="tc1")
            nc.scalar.activation(out=tc1, in_=c1, func=AF.Tanh)
            h1 = small.tile([128, 1], f32, name="h1", tag="h1")
            nc.vector.tensor_mul(h1, sig_o, tc1)
            nc.sync.dma_start(out=h1_d, in_=h1)
            nc.sync.dma_start(out=c1_d, in_=c1)

            # h1 -> f16, AllGather shards, broadcast full h1 to partitions
            h1w = small.tile([128, 1], f16, name="h1w", tag="h1w")
            nc.vector.tensor_copy(h1w, h1)
            h1_send = dpool.tile([128, 1], f16, name="h1_send", tag="h1_send")
            nc.sync.dma_start(out=h1_send, in_=h1w)
            h1_all = dpool.tile([1, NCORES * 128], f16, name="h1_all",
                                tag="h1_all")
            nc.gpsimd.collective_compute(
                "AllGather", mybir.AluOpType.bypass, replica_groups=rg,
                ins=[h1_send.opt()], outs=[h1_all.opt()])
            h1b = consts.tile([128, H], f16, name="h1b", tag="h1b")
            nc.sync.dma_start(out=h1b, in_=h1_all.to_broadcast([128, H]))

            # logits shard: fp16 W_out stream; DVE multiplies then folds the
            # 1024-wide product to 512 with one add, so ScalarE's accum (no
            # 16-bit speedup) only reads half — balances DVE/ACT under the
            # DMA rate.
            lraw = consts.tile([128, VTILES], f32, name="lraw", tag="lraw")
            for n in range(VTILES):
                wt = wpool.tile([128, H], f16, name=f"wt{n}", tag="wt")
                nc.sync.dma_start(out=wt, in_=wout_d[n * 128:(n + 1) * 128, :])
                ps = scratch.tile([128, H], f16, name=f"wprod{n}", tag="wprod")
                nc.vector.tensor_mul(ps, wt, h1b)
                ps2 = scratch.tile([128, H // 2], f16, name=f"wh{n}",
                                   tag="whalf")
                nc.vector.tensor_add(ps2, ps[:, 0:H // 2], ps[:, H // 2:H])
                nc.scalar.activation(out=ps2, in_=ps2, func=AF.Identity,
                                     accum_out=lraw[:, n:n + 1])
            logits = consts.tile([128, VTILES], f32, name="logits",
                                 tag="logits")
            nc.vector.tensor_add(logits, lraw, bout)

            # softmax denominator; cross-partition sum via PE ones-matmul
            e = scratch.tile([128, VTILES], f32, name="e_t", tag="e_t")
            s_p = small.tile([128, 1], f32, name="s_p", tag="s_p")
            nc.scalar.activation(out=e, in_=logits, func=AF.Exp,
                                 accum_out=s_p)
            s_psum = psum.tile([1, 1], f32, name="s_psum", tag="s_psum")
            nc.tensor.matmul(s_psum, s_p, ones)
            S_loc = small.tile([1, 1], f32, name="S_loc", tag="S_loc")
            nc.scalar.copy(out=S_loc, in_=s_psum)

            st_send = dpool.tile([1, 1], f32, name="st_send", tag="st_send")
            nc.sync.dma_start(out=st_send, in_=S_loc)
            st_all = dpool.tile([1, NCORES], f32, name="st_all", tag="st_all")
            nc.gpsimd.collective_compute(
                "AllGather", mybir.AluOpType.bypass, replica_groups=rg,
                ins=[st_send.opt()], outs=[st_all.opt()])
            sa = small.tile([128, NCORES], f32, name="sa", tag="sa")
            nc.sync.dma_start(out=sa, in_=st_all.to_broadcast([128, NCORES]))
            S_g = small.tile([128, 1], f32, name="S_g", tag="S_g")
            nc.vector.reduce_sum(out=S_g, in_=sa, axis=X)
            logZ = small.tile([128, 1], f32, name="logZ", tag="logZ")
            nc.scalar.activation(out=logZ, in_=S_g, func=AF.Ln)
            outn = scratch.tile([128, VTILES], f32, name="outn", tag="outn")
            nc.vector.tensor_scalar(out=outn, in0=logits, scalar1=logZ,
                                    scalar2=None,
                                    op0=mybir.AluOpType.subtract)
            nc.sync.dma_start(out=out_d, in_=outn)

    nc.compile()
    return nc


def _prep_in_maps(inputs):
    token = int(np.asarray(inputs["token"]).reshape(-1)[0])
    emb = np.asarray(inputs["emb"], dtype=np.float32)
    h0 = np.asarray(inputs["h0"], dtype=np.float32).reshape(-1)
    c0 = np.asarray(inputs["c0"], dtype=np.float32).reshape(-1)
    W_ih = np.asarray(inputs["W_ih"], dtype=np.float32).reshape(4, H, H)
    W_hh = np.asarray(inputs["W_hh"], dtype=np.float32).reshape(4, H, H)
    b = (np.asarray(inputs["b_ih"], dtype=np.float32)
         + np.asarray(inputs["b_hh"], dtype=np.float32)).reshape(4, H)
    W_out = np.asarray(inputs["W_out"], dtype=np.float32)
    b_out = np.asarray(inputs["b_out"], dtype=np.float32)

    xh = np.concatenate([emb[token], h0])[None, :]  # [1, 2H]

    in_maps = []
    for k in range(NCORES):
        lo, hi = k * 128, (k + 1) * 128
        wcat = np.concatenate([W_ih[:, lo:hi, :], W_hh[:, lo:hi, :]],
                              axis=2)               # [4, 128, 2H]
        bsum = np.ascontiguousarray(b[:, lo:hi].T)  # [128, 4]
        cblk = c0[lo:hi].reshape(128, 1)

        vlo = k * VPC
        vhi = min(vlo + VPC, V)
        nv = vhi - vlo
        wout = np.zeros((VPAD, H), dtype=np.float16)
        wout[:nv] = W_out[vlo:vhi].astype(np.float16)
        bpad = np.full(VPAD, NEG, dtype=np.float32)
        bpad[:nv] = b_out[vlo:vhi]
        bout = np.ascontiguousarray(bpad.reshape(VTILES, 128).T)  # [128, VT]

        in_maps.append({
            "xh": np.ascontiguousarray(xh),
            "wcat": np.ascontiguousarray(wcat),
            "bsum": bsum,
            "cblk": np.ascontiguousarray(cblk),
            "wout": wout,
            "bout": bout,
        })
    return in_maps


def kernel(**inputs):
    global _NC, LAST_RESULTS
    if _NC is None:
        _NC = _build_nc()

    from concourse.bass_utils import run_bass_kernel_spmd

    in_maps = _prep_in_maps(inputs)
    res = run_bass_kernel_spmd(_NC, in_maps, core_ids=list(range(NCORES)))
    LAST_RESULTS = res

    out = np.empty((1, V), dtype=np.float32)
    h1 = np.empty(NCORES * 128, dtype=np.float32)
    c1 = np.empty(NCORES * 128, dtype=np.float32)
    for k in range(NCORES):
        r = res.results[k]
        vlo = k * VPC
        vhi = min(vlo + VPC, V)
        # out_shard[p, n] holds vocab row n*128 + p of this core's shard
        out[0, vlo:vhi] = r["out_shard"].T.reshape(-1)[:vhi - vlo]
        h1[k * 128:(k + 1) * 128] = r["h1_out"].reshape(-1)
        c1[k * 128:(k + 1) * 128] = r["c1_out"].reshape(-1)

    return out, h1.reshape(1, 1, H), c1.reshape(1, 1, H)
